# revision 15
# baseline (speedup 1.0000x reference)
"""Trainium2 Bass kernel for BatchPPRFeatures:
    out[i] = sum_k ppr_scores[i,k] * x[ppr_idx[i,k]]   (N=100000, D=128, K=32)

Strategy (8 NeuronCores, node-parallel):
- Shard output rows across 8 cores (12500 rows/core, padded to 12544 = 98
  tiles of 128). x (fp16) is replicated to every core.
- The gather runs via gpsimd dma_gather (SWDGE) with int16 indices; x is
  split into 4 chunks of 25000 rows; each output tile's 4096 (i,k) entries
  are bucketed by chunk on the host into fixed-capacity segments (CAP slots,
  padded with index 0 / score 0), sorted ascending for DRAM locality.
  Q7 descriptor generation (~2ns/idx, cluster-serialized) is the bottleneck,
  so the Pool engine must never stall:
- ALL index data is preloaded into SBUF once at kernel start. The 4 chunk
  calls of a group are band-packed: queue c's ucode pair (cpus 2c, 2c+1)
  reads partitions 32c..32c+31, so chunk c's wrapped index list lives in
  that 32-partition band (16-row list duplicated twice).
- Gathered slots land as [slot%128 -> partition, slot//128 -> block]. The
  weighted reduction is per-tile accumulating PSUM matmuls with one-hot
  scatter matrices W[p, m] = score if slot(b,p) targets out-row m.
- W is built ON-CHIP by the (otherwise idle) DVE so the DMA engines stay
  free for gather-descriptor drain: host sends per-slot target row m* and
  score, each replicated 16x along the free dim ([128, NBLOCKS*16] f16).
  Per tile: onehot = (iota128 == t*) then *= s*, where the broadcast of
  t*/s* over the 8 16-column repeats uses a stride-0 MIDDLE axis
  ([16,36],[0,8],[1,16]) so every operand keeps a packed last axis and the
  DVE runs in 2x mode.
"""

import sys

sys.path.insert(0, "/opt/trn_rl_repo")

import numpy as np

N = 100000
D = 128
K = 32
N_CORES = 8
N_CHUNKS = 4
CHUNK = N // N_CHUNKS            # 25000 rows per chunk (int16-addressable)
ROWS_PER_CORE = N // N_CORES     # 12500
CAP = 1152                       # slots per (tile, chunk) segment, mult of 128
BLOCKS_SEG = CAP // 128          # 9 blocks per segment
TILES = 98                       # ceil(12500/128)
ROWS_PAD = TILES * 128           # 12544
GROUP = 4                        # tiles per gather call group
GROUP_SIZES = [GROUP] * (TILES // GROUP) + ([TILES % GROUP] if TILES % GROUP else [])
BLOCKS_TILE = N_CHUNKS * BLOCKS_SEG          # 36 blocks per tile
NBLOCKS = TILES * BLOCKS_TILE                # total one-hot blocks per core
IDX_COLS = sum(g * CAP // 16 for g in GROUP_SIZES)
REP = 1                                      # t*/s*: one value per (lane, block)

_prog_cache = {}


def _build_program():
    """Build + compile the (input-independent) SPMD Bass program."""
    if "nc" in _prog_cache:
        return _prog_cache["nc"]
    from concourse import bacc, mybir, tile
    from concourse.ap import AP

    F16 = mybir.dt.float16
    F32 = mybir.dt.float32
    I16 = mybir.dt.int16
    Alu = mybir.AluOpType

    nc = bacc.Bacc(
        "TRN2",
        target_bir_lowering=False,
        debug=False,
        num_devices=N_CORES,
        num_swdge_queues=4,
    )
    x_d = nc.dram_tensor("x", [N, D], F16, kind="ExternalInput")
    idx_d = nc.dram_tensor("idx16", [128, IDX_COLS], I16, kind="ExternalInput")
    t_d = nc.dram_tensor("t16", [128, NBLOCKS * REP], F16, kind="ExternalInput")
    s_d = nc.dram_tensor("s16", [128, NBLOCKS * REP], F16, kind="ExternalInput")
    iota_d = nc.dram_tensor("iota", [128, 128], F16, kind="ExternalInput")
    out_d = nc.dram_tensor("out", [ROWS_PAD, D], F32, kind="ExternalOutput")

    def ap3(base, axes, extra_off=0):
        """Raw 3D AP over base with given free axes and element offset."""
        return AP(
            tensor=base.tensor,
            offset=base.offset + extra_off,
            ap=[list(base.ap[0])] + axes,
        )

    with tile.TileContext(nc) as tc:
        with (
            tc.tile_pool(name="idxp", bufs=1) as idxp,
            tc.tile_pool(name="gp", bufs=2) as gpool,
            tc.tile_pool(name="ohp", bufs=2) as ohpool,
            tc.tile_pool(name="rp", bufs=2) as rpool,
            tc.tile_pool(name="tsp", bufs=3) as tspool,
            tc.tile_pool(name="op", bufs=4) as opool,
            tc.tile_pool(name="ps", bufs=4, space="PSUM") as pspool,
        ):
            # preload index table, t*/s* tables and the iota pattern once
            idx_sb = idxp.tile([128, IDX_COLS], I16, tag="idx")
            nc.scalar.dma_start(out=idx_sb[:], in_=idx_d[:, :])
            iota_sb = idxp.tile([128, 128], F16, tag="iota")
            nc.scalar.dma_start(out=iota_sb[:], in_=iota_d[:, :])
            # full t*/s* tables fit in SBUF at REP=1 (7KB/partition each)
            t_sb = idxp.tile([128, NBLOCKS * REP], F16, tag="t")
            nc.sync.dma_start(out=t_sb[:], in_=t_d[:, :])
            s_sb = idxp.tile([128, NBLOCKS * REP], F16, tag="s")
            nc.sync.dma_start(out=s_sb[:], in_=s_d[:, :])


            col = 0
            T0 = 0
            for g, gsz in enumerate(GROUP_SIZES):
                w = gsz * CAP // 16
                TW = BLOCKS_TILE * REP
                gs = []
                for c in range(N_CHUNKS):
                    g_sb = gpool.tile([128, gsz * BLOCKS_SEG * D], F16, tag=f"g{c}")
                    nc.gpsimd.dma_gather(
                        out_ap=g_sb[:].rearrange("p (b d) -> p b d", d=D),
                        in_ap=x_d[c * CHUNK : (c + 1) * CHUNK, :],
                        idxs_ap=idx_sb[:, col : col + w],
                        num_idxs=gsz * CAP,
                        num_idxs_reg=gsz * CAP,
                        elem_size=D,
                        single_packet=False,
                        queue_num=c,
                    )
                    gs.append(g_sb)
                col += w

                for t in range(gsz):
                    T = T0 + t
                    oh_sb = ohpool.tile([128, BLOCKS_TILE * 128], F16, tag="oh")
                    oh3 = oh_sb[:].rearrange("p (gb m) -> p gb m", m=128)
                    iota3 = ap3(iota_sb[:], [[0, BLOCKS_TILE], [1, 128]])
                    t3 = ap3(t_sb[:], [[1, BLOCKS_TILE], [0, 128]], T * BLOCKS_TILE)
                    s3 = ap3(s_sb[:], [[1, BLOCKS_TILE], [0, 128]], T * BLOCKS_TILE)
                    nc.vector.tensor_tensor(
                        out=oh3, in0=iota3, in1=t3, op=Alu.is_equal
                    )
                    nc.vector.tensor_tensor(
                        out=oh3, in0=oh3, in1=s3, op=Alu.mult
                    )
                    ps = pspool.tile([128, D], F32, space="PSUM")
                    nb = 0
                    for c in range(N_CHUNKS):
                        for b in range(BLOCKS_SEG):
                            gb = c * BLOCKS_SEG + b
                            nc.tensor.matmul(
                                out=ps[:],
                                lhsT=oh_sb[:, gb * 128 : (gb + 1) * 128],
                                rhs=gs[c][
                                    :,
                                    (t * BLOCKS_SEG + b) * D : (t * BLOCKS_SEG + b + 1)
                                    * D,
                                ],
                                start=(nb == 0),
                                stop=(nb == BLOCKS_TILE - 1),
                            )
                            nb += 1
                    o_sb = opool.tile([128, D], F32, tag="o")
                    nc.scalar.copy(out=o_sb[:], in_=ps[:])
                    nc.scalar.dma_start(
                        out=out_d[T * 128 : (T + 1) * 128, :], in_=o_sb[:]
                    )
                T0 += gsz

    nc.compile()
    _prog_cache["nc"] = nc
    return nc


def _prep_core_inputs(idx_core, sc_core):
    """Bucket one core's (padded) indices by chunk into fixed-cap segments.

    idx_core: [ROWS_PAD, K] int64, sc_core: [ROWS_PAD, K] float32.
    Returns (idx16 [128, IDX_COLS] int16,
             t16 [128, NBLOCKS*REP] f16, s16 [128, NBLOCKS*REP] f16).
    """
    seg_idx = np.zeros((TILES, N_CHUNKS, CAP), dtype=np.int16)
    seg_tc = np.zeros((TILES, N_CHUNKS, CAP), dtype=np.float16)
    seg_sp = np.zeros((TILES, N_CHUNKS, CAP), dtype=np.float16)

    idx_t = idx_core.reshape(TILES, 128 * K)
    sc_t = sc_core.reshape(TILES, 128 * K)
    chunk_t = idx_t // CHUNK
    p_of_e = np.arange(128 * K) // K  # target out-row of entry

    for T in range(TILES):
        ch = chunk_t[T]
        order = np.argsort(ch * N + idx_t[T], kind="stable")
        ch_s = ch[order]
        bounds = np.searchsorted(ch_s, np.arange(N_CHUNKS + 1))
        for c in range(N_CHUNKS):
            sel = order[bounds[c] : bounds[c + 1]]
            n = len(sel)
            if n > CAP:
                raise OverflowError(
                    f"segment overflow tile={T} chunk={c} n={n} > CAP={CAP}"
                )
            seg_idx[T, c, :n] = (idx_t[T, sel] - c * CHUNK).astype(np.int16)
            seg_tc[T, c, :n] = p_of_e[sel]
            seg_sp[T, c, :n] = sc_t[T, sel]

    # band-packed gather index lists: per group g, chunk c, the wrapped
    # [16, w] list goes into partitions 32c..32c+15 and 32c+16..32c+31
    idx16 = np.zeros((128, IDX_COLS), dtype=np.int16)
    col = 0
    T0 = 0
    for gsz in GROUP_SIZES:
        w = gsz * CAP // 16
        for c in range(N_CHUNKS):
            flat = seg_idx[T0 : T0 + gsz, c, :].reshape(gsz * CAP)
            wrapped = flat.reshape(w, 16).T  # [16, w]
            idx16[32 * c : 32 * c + 16, col : col + w] = wrapped
            idx16[32 * c + 16 : 32 * c + 32, col : col + w] = wrapped
        col += w
        T0 += gsz

    # per-slot target row / score, REP-replicated along free dim:
    # block gb = (T*N_CHUNKS + c)*BLOCKS_SEG + b holds slots b*128+p of
    # segment (T, c); lane p -> t16[p, gb*REP : (gb+1)*REP] = target row
    tc_blocks = seg_tc.reshape(TILES, N_CHUNKS, BLOCKS_SEG, 128).transpose(
        3, 0, 1, 2
    ).reshape(128, NBLOCKS)
    sp_blocks = seg_sp.reshape(TILES, N_CHUNKS, BLOCKS_SEG, 128).transpose(
        3, 0, 1, 2
    ).reshape(128, NBLOCKS)
    t16 = tc_blocks.copy()
    s16 = sp_blocks.copy()

    return (
        np.ascontiguousarray(idx16),
        np.ascontiguousarray(t16),
        np.ascontiguousarray(s16),
    )


def make_in_maps(x, ppr_idx, ppr_scores):
    x16 = np.asarray(x).astype(np.float16)
    ppr_idx = np.asarray(ppr_idx)
    ppr_scores = np.asarray(ppr_scores)

    idx_pad = np.zeros((N_CORES, ROWS_PAD, K), dtype=np.int64)
    sc_pad = np.zeros((N_CORES, ROWS_PAD, K), dtype=np.float32)
    # spread zero-weight padding rows' indices across chunks so no
    # per-(tile, chunk) segment overflows its fixed capacity
    idx_pad[:, ROWS_PER_CORE:] = (np.arange(K) % N_CHUNKS) * CHUNK
    idx_pad[:, :ROWS_PER_CORE] = ppr_idx.reshape(N_CORES, ROWS_PER_CORE, K)
    sc_pad[:, :ROWS_PER_CORE] = ppr_scores.reshape(N_CORES, ROWS_PER_CORE, K)

    iota = np.broadcast_to(
        np.arange(128, dtype=np.float16)[None, :], (128, 128)
    ).copy()

    in_maps = []
    for c in range(N_CORES):
        idx16, t16, s16 = _prep_core_inputs(idx_pad[c], sc_pad[c])
        in_maps.append(
            {"x": x16, "idx16": idx16, "t16": t16, "s16": s16, "iota": iota}
        )
    return in_maps


def kernel(x, ppr_idx, ppr_scores):
    from concourse.bass_utils import run_bass_kernel_spmd

    nc = _build_program()
    in_maps = make_in_maps(x, ppr_idx, ppr_scores)
    res = run_bass_kernel_spmd(nc, in_maps, core_ids=list(range(N_CORES)))
    out = np.concatenate(
        [res.results[c]["out"][:ROWS_PER_CORE] for c in range(N_CORES)], axis=0
    )
    return out.astype(np.float32)


# revision 16
# speedup vs baseline: 1.1307x; 1.1307x over previous
"""Trainium2 Bass kernel for BatchPPRFeatures:
    out[i] = sum_k ppr_scores[i,k] * x[ppr_idx[i,k]]   (N=100000, D=128, K=32)

Strategy (8 NeuronCores, node-parallel):
- Shard output rows across 8 cores (12500 rows/core, padded to 12544 = 98
  tiles of 128). x (fp16) is replicated to every core.
- The gather runs via gpsimd dma_gather (SWDGE) with int16 indices; x is
  split into 4 chunks of 25000 rows; each output tile's 4096 (i,k) entries
  are bucketed by chunk on the host into fixed-capacity segments (CAP slots,
  padded with index 0 / score 0), sorted ascending for DRAM locality.
  Q7 descriptor generation (~2ns/idx, cluster-serialized) is the bottleneck,
  so the Pool engine must never stall:
- ALL index data is preloaded into SBUF once at kernel start. The 4 chunk
  calls of a group are band-packed: queue c's ucode pair (cpus 2c, 2c+1)
  reads partitions 32c..32c+31, so chunk c's wrapped index list lives in
  that 32-partition band (16-row list duplicated twice).
- Gathered slots land as [slot%128 -> partition, slot//128 -> block]. The
  weighted reduction is per-tile accumulating PSUM matmuls with one-hot
  scatter matrices W[p, m] = score if slot(b,p) targets out-row m.
- W is built ON-CHIP by the (otherwise idle) DVE so the DMA engines stay
  free for gather-descriptor drain: host sends per-slot target row m* and
  score, each replicated 16x along the free dim ([128, NBLOCKS*16] f16).
  Per tile: onehot = (iota128 == t*) then *= s*, where the broadcast of
  t*/s* over the 8 16-column repeats uses a stride-0 MIDDLE axis
  ([16,36],[0,8],[1,16]) so every operand keeps a packed last axis and the
  DVE runs in 2x mode.
"""

import sys

sys.path.insert(0, "/opt/trn_rl_repo")

import numpy as np

N = 100000
D = 128
K = 32
N_CORES = 8
N_CHUNKS = 4
CHUNK = N // N_CHUNKS            # 25000 rows per chunk (int16-addressable)
ROWS_PER_CORE = N // N_CORES     # 12500
CAP = 1152                       # slots per (tile, chunk) segment, mult of 128
BLOCKS_SEG = CAP // 128          # 9 blocks per segment
TILES = 98                       # ceil(12500/128)
ROWS_PAD = TILES * 128           # 12544
GROUP = 4                        # tiles per gather call group
GROUP_SIZES = [GROUP] * (TILES // GROUP) + ([TILES % GROUP] if TILES % GROUP else [])
BLOCKS_TILE = N_CHUNKS * BLOCKS_SEG          # 36 blocks per tile
NBLOCKS = TILES * BLOCKS_TILE                # total one-hot blocks per core
IDX_COLS = sum(g * CAP // 16 for g in GROUP_SIZES)
REP = 1                                      # t*/s*: one value per (lane, block)

_prog_cache = {}


def _build_program():
    """Build + compile the (input-independent) SPMD Bass program."""
    if "nc" in _prog_cache:
        return _prog_cache["nc"]
    from concourse import bacc, mybir, tile
    from concourse.ap import AP

    F16 = mybir.dt.float16
    F32 = mybir.dt.float32
    I16 = mybir.dt.int16
    Alu = mybir.AluOpType

    nc = bacc.Bacc(
        "TRN2",
        target_bir_lowering=False,
        debug=False,
        num_devices=N_CORES,
        num_swdge_queues=4,
    )
    x_d = nc.dram_tensor("x", [N, D], F16, kind="ExternalInput")
    idx_d = nc.dram_tensor("idx16", [128, IDX_COLS], I16, kind="ExternalInput")
    t_d = nc.dram_tensor("t16", [128, NBLOCKS * REP], F16, kind="ExternalInput")
    s_d = nc.dram_tensor("s16", [128, NBLOCKS * REP], F16, kind="ExternalInput")
    iota_d = nc.dram_tensor("iota", [128, 128], F16, kind="ExternalInput")
    out_d = nc.dram_tensor("out", [ROWS_PAD, D], F32, kind="ExternalOutput")

    def ap3(base, axes, extra_off=0):
        """Raw 3D AP over base with given free axes and element offset."""
        return AP(
            tensor=base.tensor,
            offset=base.offset + extra_off,
            ap=[list(base.ap[0])] + axes,
        )

    with tile.TileContext(nc) as tc:
        with (
            tc.tile_pool(name="idxp", bufs=1) as idxp,
            tc.tile_pool(name="gp", bufs=2) as gpool,
            tc.tile_pool(name="ohp", bufs=2) as ohpool,
            tc.tile_pool(name="rp", bufs=2) as rpool,
            tc.tile_pool(name="tsp", bufs=3) as tspool,
            tc.tile_pool(name="op", bufs=4) as opool,
            tc.tile_pool(name="ps", bufs=4, space="PSUM") as pspool,
        ):
            # preload index table, t*/s* tables and the iota pattern once
            idx_sb = idxp.tile([128, IDX_COLS], I16, tag="idx")
            nc.scalar.dma_start(out=idx_sb[:], in_=idx_d[:, :])
            iota_sb = idxp.tile([128, 128], F16, tag="iota")
            nc.scalar.dma_start(out=iota_sb[:], in_=iota_d[:, :])
            # full t*/s* tables fit in SBUF at REP=1 (7KB/partition each)
            t_sb = idxp.tile([128, NBLOCKS * REP], F16, tag="t")
            nc.sync.dma_start(out=t_sb[:], in_=t_d[:, :])
            s_sb = idxp.tile([128, NBLOCKS * REP], F16, tag="s")
            nc.sync.dma_start(out=s_sb[:], in_=s_d[:, :])


            col = 0
            T0 = 0
            for g, gsz in enumerate(GROUP_SIZES):
                w = gsz * CAP // 16
                TW = BLOCKS_TILE * REP
                gs = []
                for c in range(N_CHUNKS):
                    g_sb = gpool.tile([128, gsz * BLOCKS_SEG * D], F16, tag=f"g{c}")
                    nc.gpsimd.dma_gather(
                        out_ap=g_sb[:].rearrange("p (b d) -> p b d", d=D),
                        in_ap=x_d[c * CHUNK : (c + 1) * CHUNK, :],
                        idxs_ap=idx_sb[:, col : col + w],
                        num_idxs=gsz * CAP,
                        num_idxs_reg=gsz * CAP,
                        elem_size=D,
                        single_packet=False,
                        queue_num=c,
                    )
                    gs.append(g_sb)
                col += w

                for t in range(gsz):
                    T = T0 + t
                    oh_sb = ohpool.tile([128, BLOCKS_TILE * 128], F16, tag="oh")
                    oh3 = oh_sb[:].rearrange("p (gb m) -> p gb m", m=128)
                    iota3 = ap3(iota_sb[:], [[0, BLOCKS_TILE], [1, 128]])
                    t3 = ap3(t_sb[:], [[1, BLOCKS_TILE], [0, 128]], T * BLOCKS_TILE)
                    s3 = ap3(s_sb[:], [[1, BLOCKS_TILE], [0, 128]], T * BLOCKS_TILE)
                    # ACT pre-expands t*/s* to packed [128, 36*128] so both
                    # DVE ops run with unit-stride operands (fast DVE mode)
                    tr_sb = rpool.tile([128, BLOCKS_TILE * 128], F16, tag="tr")
                    tr3 = tr_sb[:].rearrange("p (gb m) -> p gb m", m=128)
                    nc.scalar.copy(out=tr3, in_=t3)
                    sr_sb = rpool.tile([128, BLOCKS_TILE * 128], F16, tag="sr")
                    sr3 = sr_sb[:].rearrange("p (gb m) -> p gb m", m=128)
                    nc.scalar.copy(out=sr3, in_=s3)
                    nc.vector.tensor_tensor(
                        out=oh3, in0=iota3, in1=tr3, op=Alu.is_equal
                    )
                    nc.vector.tensor_tensor(
                        out=oh3, in0=oh3, in1=sr3, op=Alu.mult
                    )
                    ps = pspool.tile([128, D], F32, space="PSUM")
                    nb = 0
                    for c in range(N_CHUNKS):
                        for b in range(BLOCKS_SEG):
                            gb = c * BLOCKS_SEG + b
                            nc.tensor.matmul(
                                out=ps[:],
                                lhsT=oh_sb[:, gb * 128 : (gb + 1) * 128],
                                rhs=gs[c][
                                    :,
                                    (t * BLOCKS_SEG + b) * D : (t * BLOCKS_SEG + b + 1)
                                    * D,
                                ],
                                start=(nb == 0),
                                stop=(nb == BLOCKS_TILE - 1),
                            )
                            nb += 1
                    o_sb = opool.tile([128, D], F32, tag="o")
                    nc.scalar.copy(out=o_sb[:], in_=ps[:])
                    nc.scalar.dma_start(
                        out=out_d[T * 128 : (T + 1) * 128, :], in_=o_sb[:]
                    )
                T0 += gsz

    nc.compile()
    _prog_cache["nc"] = nc
    return nc


def _prep_core_inputs(idx_core, sc_core):
    """Bucket one core's (padded) indices by chunk into fixed-cap segments.

    idx_core: [ROWS_PAD, K] int64, sc_core: [ROWS_PAD, K] float32.
    Returns (idx16 [128, IDX_COLS] int16,
             t16 [128, NBLOCKS*REP] f16, s16 [128, NBLOCKS*REP] f16).
    """
    seg_idx = np.zeros((TILES, N_CHUNKS, CAP), dtype=np.int16)
    seg_tc = np.zeros((TILES, N_CHUNKS, CAP), dtype=np.float16)
    seg_sp = np.zeros((TILES, N_CHUNKS, CAP), dtype=np.float16)

    idx_t = idx_core.reshape(TILES, 128 * K)
    sc_t = sc_core.reshape(TILES, 128 * K)
    chunk_t = idx_t // CHUNK
    p_of_e = np.arange(128 * K) // K  # target out-row of entry

    for T in range(TILES):
        ch = chunk_t[T]
        order = np.argsort(ch * N + idx_t[T], kind="stable")
        ch_s = ch[order]
        bounds = np.searchsorted(ch_s, np.arange(N_CHUNKS + 1))
        for c in range(N_CHUNKS):
            sel = order[bounds[c] : bounds[c + 1]]
            n = len(sel)
            if n > CAP:
                raise OverflowError(
                    f"segment overflow tile={T} chunk={c} n={n} > CAP={CAP}"
                )
            seg_idx[T, c, :n] = (idx_t[T, sel] - c * CHUNK).astype(np.int16)
            seg_tc[T, c, :n] = p_of_e[sel]
            seg_sp[T, c, :n] = sc_t[T, sel]

    # band-packed gather index lists: per group g, chunk c, the wrapped
    # [16, w] list goes into partitions 32c..32c+15 and 32c+16..32c+31
    idx16 = np.zeros((128, IDX_COLS), dtype=np.int16)
    col = 0
    T0 = 0
    for gsz in GROUP_SIZES:
        w = gsz * CAP // 16
        for c in range(N_CHUNKS):
            flat = seg_idx[T0 : T0 + gsz, c, :].reshape(gsz * CAP)
            wrapped = flat.reshape(w, 16).T  # [16, w]
            idx16[32 * c : 32 * c + 16, col : col + w] = wrapped
            idx16[32 * c + 16 : 32 * c + 32, col : col + w] = wrapped
        col += w
        T0 += gsz

    # per-slot target row / score, REP-replicated along free dim:
    # block gb = (T*N_CHUNKS + c)*BLOCKS_SEG + b holds slots b*128+p of
    # segment (T, c); lane p -> t16[p, gb*REP : (gb+1)*REP] = target row
    tc_blocks = seg_tc.reshape(TILES, N_CHUNKS, BLOCKS_SEG, 128).transpose(
        3, 0, 1, 2
    ).reshape(128, NBLOCKS)
    sp_blocks = seg_sp.reshape(TILES, N_CHUNKS, BLOCKS_SEG, 128).transpose(
        3, 0, 1, 2
    ).reshape(128, NBLOCKS)
    t16 = tc_blocks.copy()
    s16 = sp_blocks.copy()

    return (
        np.ascontiguousarray(idx16),
        np.ascontiguousarray(t16),
        np.ascontiguousarray(s16),
    )


def make_in_maps(x, ppr_idx, ppr_scores):
    x16 = np.asarray(x).astype(np.float16)
    ppr_idx = np.asarray(ppr_idx)
    ppr_scores = np.asarray(ppr_scores)

    idx_pad = np.zeros((N_CORES, ROWS_PAD, K), dtype=np.int64)
    sc_pad = np.zeros((N_CORES, ROWS_PAD, K), dtype=np.float32)
    # spread zero-weight padding rows' indices across chunks so no
    # per-(tile, chunk) segment overflows its fixed capacity
    idx_pad[:, ROWS_PER_CORE:] = (np.arange(K) % N_CHUNKS) * CHUNK
    idx_pad[:, :ROWS_PER_CORE] = ppr_idx.reshape(N_CORES, ROWS_PER_CORE, K)
    sc_pad[:, :ROWS_PER_CORE] = ppr_scores.reshape(N_CORES, ROWS_PER_CORE, K)

    iota = np.broadcast_to(
        np.arange(128, dtype=np.float16)[None, :], (128, 128)
    ).copy()

    in_maps = []
    for c in range(N_CORES):
        idx16, t16, s16 = _prep_core_inputs(idx_pad[c], sc_pad[c])
        in_maps.append(
            {"x": x16, "idx16": idx16, "t16": t16, "s16": s16, "iota": iota}
        )
    return in_maps


def kernel(x, ppr_idx, ppr_scores):
    from concourse.bass_utils import run_bass_kernel_spmd

    nc = _build_program()
    in_maps = make_in_maps(x, ppr_idx, ppr_scores)
    res = run_bass_kernel_spmd(nc, in_maps, core_ids=list(range(N_CORES)))
    out = np.concatenate(
        [res.results[c]["out"][:ROWS_PER_CORE] for c in range(N_CORES)], axis=0
    )
    return out.astype(np.float32)
